# revision 8
# baseline (speedup 1.0000x reference)
"""GateRow kernel for Trainium2 (8 NeuronCores, SPMD gate-parallel).

Problem: out[b, g] = gates[g, 2*x[b, c0[g]] + x[b, c1[g]]]
  x: [16384, 8192] bool, gates: [8192, 4] bool, choices: [8192, 2] int32.

Strategy: bit-pack the batch dimension (8 rows/byte; stored as uint16
words for 2x DVE throughput) so every boolean gate evaluates bitwise.
Every 2-input boolean function is either a single table row (copies,
constants, inverses -- the table holds x, ~x, zeros, ones) or  P op Q
with op in {AND, OR, XOR} and P, Q table rows.  Gates are sharded
across the 8 cores and sorted by op-class into 128-gate blocks so each
core runs one bitwise tensor_tensor per AND/OR/XOR block and nothing at
all for COPY blocks (gathered rows stream straight back out).

Per core: dma_gather ~1.7k rows of 2048B (~3.3 MiB), ~5 DVE bitwise
ops over [128, 1024] uint16 tiles, DMA out 2 MiB of packed results.
Host side: pack bits, build the table, classify/sort gates, unpack +
transpose the packed output.
"""

import sys

for _p in ("/opt/trn_rl_repo", "/opt/pypackages"):
    if _p not in sys.path:
        sys.path.append(_p)

from contextlib import ExitStack

import numpy as np

import concourse.bacc as bacc
import concourse.tile as tile
import concourse.mybir as mybir
from concourse.bass_utils import run_bass_kernel_spmd

B, N, G, NCORES = 16384, 8192, 8192, 8
GPC = G // NCORES          # 1024 gate slots per core
PB = B // 8                # 2048 packed bytes per table row
PW = PB // 2               # 1024 uint16 words per table row
NTAB = 2 * N + 2           # x rows, ~x rows, zeros row, ones row
NBLK = GPC // 128          # 8 blocks of 128 gates per core

# ---------------------------------------------------------------------------
# Gate classification.  Truth table tt (bit i = gates[g, i], i = 2a+b).
# Classes: 0:AND  1:OR  2:XOR (two rows)   3:COPY (single row).
# Sections: 0:x[c0] 1:~x[c0] 2:x[c1] 3:~x[c1] 4:zeros 5:ones.
# ---------------------------------------------------------------------------


def _forms():
    forms = [[None] * 4 for _ in range(16)]
    for tt in range(16):
        for cls in range(3):
            for ps in range(6):
                for qs in range(6):
                    ok = True
                    for a in (0, 1):
                        for b in (0, 1):
                            va = (a, 1 - a, b, 1 - b, 0, 1)[ps]
                            vb = (a, 1 - a, b, 1 - b, 0, 1)[qs]
                            f = (va & vb, va | vb, va ^ vb)[cls]
                            if f != ((tt >> (2 * a + b)) & 1):
                                ok = False
                    if ok and forms[tt][cls] is None:
                        forms[tt][cls] = (ps, qs)
        for ps in range(6):
            ok = all(
                (a, 1 - a, b, 1 - b, 0, 1)[ps] == ((tt >> (2 * a + b)) & 1)
                for a in (0, 1)
                for b in (0, 1)
            )
            if ok and forms[tt][3] is None:
                forms[tt][3] = (ps, ps)
    return forms


_FORMS = _forms()


def _sec_rows(sec, c0, c1):
    return np.select(
        [sec == 0, sec == 1, sec == 2, sec == 3, sec == 4, sec == 5],
        [c0, N + c0, c1, N + c1,
         np.full(sec.shape, 2 * N), np.full(sec.shape, 2 * N + 1)],
    )


# ---------------------------------------------------------------------------
# Device program.  Uniform across cores: blocks [0, na) AND, [na, na+no)
# OR, [na+no, naox) XOR, [naox, 8) COPY.  Gathers run per half (blocks
# 0-3 then 4-7): a p-call for all 4 blocks, a q-call for the AOX blocks.
# ---------------------------------------------------------------------------


def build_nc(na, no, nx):
    naox = na + no + nx
    assert naox <= NBLK
    nc = bacc.Bacc(
        "TRN2", target_bir_lowering=False, debug=False, num_devices=NCORES
    )
    ncols = NBLK * 8 + naox * 8  # int16 idx columns: p-stream then q-stream
    tab = nc.dram_tensor("tab", [NTAB, PW], mybir.dt.uint16, kind="ExternalInput")
    idxs = nc.dram_tensor("idxs", [128, ncols], mybir.dt.int16, kind="ExternalInput")
    outd = nc.dram_tensor("out", [GPC, PW], mybir.dt.uint16, kind="ExternalOutput")

    ncopy = NBLK - naox
    ops = (
        [None] * ncopy
        + [mybir.AluOpType.bitwise_and] * na
        + [mybir.AluOpType.bitwise_or] * no
        + [mybir.AluOpType.bitwise_xor] * nx
    )
    hb = NBLK // 2  # blocks per half

    with tile.TileContext(nc) as tc, ExitStack() as ctx:
        pconst = ctx.enter_context(tc.tile_pool(name="const", bufs=1))
        pdata = ctx.enter_context(tc.tile_pool(name="data", bufs=1))

        idx_t = pconst.tile([128, ncols], mybir.dt.int16)
        nc.sync.dma_start(idx_t[:], idxs[:])

        p_t = pdata.tile([128, NBLK, PW], mybir.dt.uint16)
        q_t = pdata.tile([128, max(naox, 1), PW], mybir.dt.uint16)
        lut = pdata.tile([128, max(naox, 1), PW], mybir.dt.uint16)

        nreg = {}
        for n in {hb * 128, *(
            128 * len([j for j in range(h * hb, (h + 1) * hb) if j >= ncopy])
            for h in (0, 1)
        )} - {0}:
            nreg[n] = nc.gpsimd.to_reg(n)

        for h in (0, 1):
            blocks = range(h * hb, (h + 1) * hb)
            aox = [j for j in blocks if j >= ncopy]
            npi = hb * 128
            nc.gpsimd.dma_gather(
                p_t[:, h * hb : (h + 1) * hb, :],
                tab[:],
                idx_t[:, h * (npi // 16) : (h + 1) * (npi // 16)],
                npi,
                nreg[npi],
                PW,
                single_packet=True,
            )
            if aox:
                nqi = 128 * len(aox)
                qc0 = NBLK * 8 + (aox[0] - ncopy) * 8
                nc.gpsimd.dma_gather(
                    q_t[:, aox[0] - ncopy : aox[0] - ncopy + len(aox), :],
                    tab[:],
                    idx_t[:, qc0 : qc0 + nqi // 16],
                    nqi,
                    nreg[nqi],
                    PW,
                    single_packet=True,
                )
            for j in blocks:
                if j >= ncopy:
                    nc.vector.tensor_tensor(
                        lut[:, j - ncopy, :],
                        p_t[:, j, :],
                        q_t[:, j - ncopy, :],
                        ops[j],
                    )
                    src = lut[:, j - ncopy, :]
                else:
                    src = p_t[:, j, :]
                nc.sync.dma_start(outd[j * 128 : (j + 1) * 128, :], src)
    nc.compile()
    return nc


_NC_CACHE = {}


def _get_nc(key):
    if key not in _NC_CACHE:
        _NC_CACHE[key] = build_nc(*key)
    return _NC_CACHE[key]


# ---------------------------------------------------------------------------
# Host-side planning.
# ---------------------------------------------------------------------------


def _plan(gates, choices):
    gates8 = np.asarray(gates, dtype=np.uint8)
    ch = np.asarray(choices, dtype=np.int64)
    tt = (gates8 << np.arange(4, dtype=np.uint8)).sum(axis=1).astype(np.int64)

    copyable = np.array([_FORMS[t][3] is not None for t in range(16)])[tt]
    cls_strict = np.array(
        [next(c for c in range(3) if _FORMS[t][c] is not None) for t in range(16)]
    )[tt]
    strict = [np.where(~copyable & (cls_strict == c))[0] for c in range(3)]
    copies = np.where(copyable)[0]

    # deal strict gates round-robin
    assign = [[[] for _ in range(4)] for _ in range(NCORES)]
    for c in range(3):
        for i, g in enumerate(strict[c]):
            assign[i % NCORES][c].append(g)

    maxc = [max(len(assign[k][c]) for k in range(NCORES)) for c in range(3)]
    na, no, nx = (int(np.ceil(m / 128)) for m in maxc)
    naox = na + no + nx
    assert naox <= NBLK, (na, no, nx)
    caps = [na * 128, no * 128, nx * 128]

    # copy-capable gates: pad AOX segments to caps, rest go to COPY blocks
    ci = 0
    copies = list(copies)
    for k in range(NCORES):
        for c in range(3):
            while len(assign[k][c]) < caps[c]:
                assign[k][c].append(copies[ci])
                ci += 1
        need = GPC - naox * 128
        assign[k][3] = copies[ci : ci + need]
        ci += need
    assert ci == len(copies)

    psec_tab = np.full((16, 4), -1, dtype=np.int64)
    qsec_tab = np.full((16, 4), -1, dtype=np.int64)
    for t in range(16):
        for c in range(4):
            if _FORMS[t][c] is not None:
                psec_tab[t, c], qsec_tab[t, c] = _FORMS[t][c]

    g_of_slot = np.empty((NCORES, GPC), dtype=np.int64)
    idx_maps = []
    hb = NBLK // 2
    ncopy = NBLK - naox
    for k in range(NCORES):
        segs, segcls = [], []
        for c in (3, 0, 1, 2):
            gk = np.asarray(assign[k][c], dtype=np.int64)
            if not len(gk):
                continue
            # sort by p-row for DRAM locality
            pr = _sec_rows(psec_tab[tt[gk], c], ch[gk, 0], ch[gk, 1])
            o = np.argsort(pr, kind="stable")
            segs.append(gk[o])
            segcls.append(np.full(len(gk), c))
        gk = np.concatenate(segs)
        cls = np.concatenate(segcls)
        assert gk.shape == (GPC,)
        g_of_slot[k] = gk
        p_rows = _sec_rows(psec_tab[tt[gk], cls], ch[gk, 0], ch[gk, 1])
        q_rows = _sec_rows(qsec_tab[tt[gk], cls], ch[gk, 0], ch[gk, 1])

        cols = []
        for h in (0, 1):
            flat = p_rows[h * hb * 128 : (h + 1) * hb * 128].astype(np.int16)
            cols.append(np.tile(flat.reshape(-1, 16).T, (8, 1)))
        for h in (0, 1):
            lo, hi = max(h * hb, ncopy), (h + 1) * hb
            if lo < hi:
                flat = q_rows[lo * 128 : hi * 128].astype(np.int16)
                cols.append(np.tile(flat.reshape(-1, 16).T, (8, 1)))
        idx_maps.append(np.ascontiguousarray(np.concatenate(cols, axis=1)))

    return (na, no, nx), g_of_slot, idx_maps


def _build_tab(x):
    x8 = np.asarray(x, dtype=np.uint8)
    xp = np.packbits(x8, axis=0)              # [PB, N]
    tab = np.empty((NTAB, PB), dtype=np.uint8)
    tab[:N] = xp.T
    tab[N : 2 * N] = 255 - tab[:N]
    tab[2 * N] = 0
    tab[2 * N + 1] = 255
    return tab.view(np.uint16)


# ---------------------------------------------------------------------------
# Entry point
# ---------------------------------------------------------------------------

_PLAN_CACHE = {}


def _get_plan(gates, choices):
    h = hash((gates.tobytes(), choices.tobytes()))
    if h not in _PLAN_CACHE:
        _PLAN_CACHE[h] = _plan(gates, choices)
    return _PLAN_CACHE[h]


def kernel(x, gates, choices):
    aox, g_of_slot, idx_maps = _get_plan(np.asarray(gates), np.asarray(choices))
    tab = _build_tab(x)
    nc = _get_nc(aox)
    in_maps = [{"tab": tab, "idxs": idx_maps[k]} for k in range(NCORES)]
    res = run_bass_kernel_spmd(nc, in_maps, list(range(NCORES)))

    packed = np.empty((G, PB), dtype=np.uint8)
    for k in range(NCORES):
        packed[g_of_slot[k]] = res.results[k]["out"].view(np.uint8)
    out = np.unpackbits(np.ascontiguousarray(packed.T), axis=0)
    return out.view(np.bool_)


# revision 9
# speedup vs baseline: 1.0487x; 1.0487x over previous
"""GateRow kernel for Trainium2 (8 NeuronCores, SPMD gate-parallel).

Problem: out[b, g] = gates[g, 2*x[b, c0[g]] + x[b, c1[g]]]
  x: [16384, 8192] bool, gates: [8192, 4] bool, choices: [8192, 2] int32.

Strategy: bit-pack the batch dimension (8 rows/byte; stored as uint16
words for 2x DVE throughput) so every boolean gate evaluates bitwise.
Every 2-input boolean function is either a single table row (copies,
constants, inverses -- the table holds x, ~x, zeros, ones) or  P op Q
with op in {AND, OR, XOR} and P, Q table rows.  Gates are sharded
across the 8 cores and sorted by op-class into 128-gate blocks so each
core runs one bitwise tensor_tensor per AND/OR/XOR block and nothing at
all for COPY blocks (gathered rows stream straight back out).

Per core: dma_gather ~1.7k rows of 2048B (~3.3 MiB), ~5 DVE bitwise
ops over [128, 1024] uint16 tiles, DMA out 2 MiB of packed results.
Host side: pack bits, build the table, classify/sort gates, unpack +
transpose the packed output.
"""

import sys

for _p in ("/opt/trn_rl_repo", "/opt/pypackages"):
    if _p not in sys.path:
        sys.path.append(_p)

from contextlib import ExitStack

import numpy as np

import concourse.bacc as bacc
import concourse.tile as tile
import concourse.mybir as mybir
from concourse.bass_utils import run_bass_kernel_spmd

B, N, G, NCORES = 16384, 8192, 8192, 8
GPC = G // NCORES          # 1024 gate slots per core
PB = B // 8                # 2048 packed bytes per table row
PW = PB // 2               # 1024 uint16 words per table row
NTAB = 2 * N + 2           # x rows, ~x rows, zeros row, ones row
NBLK = GPC // 128          # 8 blocks of 128 gates per core

# ---------------------------------------------------------------------------
# Gate classification.  Truth table tt (bit i = gates[g, i], i = 2a+b).
# Classes: 0:AND  1:OR  2:XOR (two rows)   3:COPY (single row).
# Sections: 0:x[c0] 1:~x[c0] 2:x[c1] 3:~x[c1] 4:zeros 5:ones.
# ---------------------------------------------------------------------------


def _forms():
    forms = [[None] * 4 for _ in range(16)]
    for tt in range(16):
        for cls in range(3):
            for ps in range(6):
                for qs in range(6):
                    ok = True
                    for a in (0, 1):
                        for b in (0, 1):
                            va = (a, 1 - a, b, 1 - b, 0, 1)[ps]
                            vb = (a, 1 - a, b, 1 - b, 0, 1)[qs]
                            f = (va & vb, va | vb, va ^ vb)[cls]
                            if f != ((tt >> (2 * a + b)) & 1):
                                ok = False
                    if ok and forms[tt][cls] is None:
                        forms[tt][cls] = (ps, qs)
        for ps in range(6):
            ok = all(
                (a, 1 - a, b, 1 - b, 0, 1)[ps] == ((tt >> (2 * a + b)) & 1)
                for a in (0, 1)
                for b in (0, 1)
            )
            if ok and forms[tt][3] is None:
                forms[tt][3] = (ps, ps)
    return forms


_FORMS = _forms()


def _sec_rows(sec, c0, c1):
    return np.select(
        [sec == 0, sec == 1, sec == 2, sec == 3, sec == 4, sec == 5],
        [c0, N + c0, c1, N + c1,
         np.full(sec.shape, 2 * N), np.full(sec.shape, 2 * N + 1)],
    )


# ---------------------------------------------------------------------------
# Device program.  Uniform across cores: blocks [0, na) AND, [na, na+no)
# OR, [na+no, naox) XOR, [naox, 8) COPY.  Gathers run per half (blocks
# 0-3 then 4-7): a p-call for all 4 blocks, a q-call for the AOX blocks.
# ---------------------------------------------------------------------------


def build_nc(na, no, nx):
    naox = na + no + nx
    assert naox <= NBLK
    nc = bacc.Bacc(
        "TRN2", target_bir_lowering=False, debug=False, num_devices=NCORES
    )
    ncols = NBLK * 8 + naox * 8  # int16 idx columns: p-stream then q-stream
    tab = nc.dram_tensor("tab", [NTAB, PW], mybir.dt.uint16, kind="ExternalInput")
    idxs = nc.dram_tensor("idxs", [128, ncols], mybir.dt.int16, kind="ExternalInput")
    outd = nc.dram_tensor("out", [GPC, PW], mybir.dt.uint16, kind="ExternalOutput")

    ops = (
        [mybir.AluOpType.bitwise_and] * na
        + [mybir.AluOpType.bitwise_or] * no
        + [mybir.AluOpType.bitwise_xor] * nx
    )
    hb = NBLK // 2  # blocks per half

    with tile.TileContext(nc) as tc, ExitStack() as ctx:
        pconst = ctx.enter_context(tc.tile_pool(name="const", bufs=1))
        pdata = ctx.enter_context(tc.tile_pool(name="data", bufs=1))

        idx_t = pconst.tile([128, ncols], mybir.dt.int16)
        nc.sync.dma_start(idx_t[:], idxs[:])

        p_t = pdata.tile([128, NBLK, PW], mybir.dt.uint16)
        q_t = pdata.tile([128, max(naox, 1), PW], mybir.dt.uint16)
        lut = pdata.tile([128, max(naox, 1), PW], mybir.dt.uint16)

        nreg = {}
        for n in {hb * 128, *(
            128 * len([j for j in range(h * hb, (h + 1) * hb) if j < naox])
            for h in (0, 1)
        )} - {0}:
            nreg[n] = nc.gpsimd.to_reg(n)

        for h in (0, 1):
            blocks = range(h * hb, (h + 1) * hb)
            aox = [j for j in blocks if j < naox]
            npi = hb * 128
            nc.gpsimd.dma_gather(
                p_t[:, h * hb : (h + 1) * hb, :],
                tab[:],
                idx_t[:, h * (npi // 16) : (h + 1) * (npi // 16)],
                npi,
                nreg[npi],
                PW,
                single_packet=True,
            )
            if aox:
                nqi = 128 * len(aox)
                qc0 = NBLK * 8 + aox[0] * 8
                nc.gpsimd.dma_gather(
                    q_t[:, aox[0] : aox[0] + len(aox), :],
                    tab[:],
                    idx_t[:, qc0 : qc0 + nqi // 16],
                    nqi,
                    nreg[nqi],
                    PW,
                    single_packet=True,
                )
            for j in blocks:
                if j < naox:
                    nc.vector.tensor_tensor(
                        lut[:, j, :], p_t[:, j, :], q_t[:, j, :], ops[j]
                    )
                    src = lut[:, j, :]
                else:
                    src = p_t[:, j, :]
                nc.sync.dma_start(outd[j * 128 : (j + 1) * 128, :], src)
    nc.compile()
    return nc


_NC_CACHE = {}


def _get_nc(key):
    if key not in _NC_CACHE:
        _NC_CACHE[key] = build_nc(*key)
    return _NC_CACHE[key]


# ---------------------------------------------------------------------------
# Host-side planning.
# ---------------------------------------------------------------------------


def _plan(gates, choices):
    gates8 = np.asarray(gates, dtype=np.uint8)
    ch = np.asarray(choices, dtype=np.int64)
    tt = (gates8 << np.arange(4, dtype=np.uint8)).sum(axis=1).astype(np.int64)

    copyable = np.array([_FORMS[t][3] is not None for t in range(16)])[tt]
    cls_strict = np.array(
        [next(c for c in range(3) if _FORMS[t][c] is not None) for t in range(16)]
    )[tt]
    strict = [np.where(~copyable & (cls_strict == c))[0] for c in range(3)]
    copies = np.where(copyable)[0]

    # deal strict gates round-robin
    assign = [[[] for _ in range(4)] for _ in range(NCORES)]
    for c in range(3):
        for i, g in enumerate(strict[c]):
            assign[i % NCORES][c].append(g)

    maxc = [max(len(assign[k][c]) for k in range(NCORES)) for c in range(3)]
    na, no, nx = (int(np.ceil(m / 128)) for m in maxc)
    naox = na + no + nx
    assert naox <= NBLK, (na, no, nx)
    caps = [na * 128, no * 128, nx * 128]

    # copy-capable gates: pad AOX segments to caps, rest go to COPY blocks
    ci = 0
    copies = list(copies)
    for k in range(NCORES):
        for c in range(3):
            while len(assign[k][c]) < caps[c]:
                assign[k][c].append(copies[ci])
                ci += 1
        need = GPC - naox * 128
        assign[k][3] = copies[ci : ci + need]
        ci += need
    assert ci == len(copies)

    psec_tab = np.full((16, 4), -1, dtype=np.int64)
    qsec_tab = np.full((16, 4), -1, dtype=np.int64)
    for t in range(16):
        for c in range(4):
            if _FORMS[t][c] is not None:
                psec_tab[t, c], qsec_tab[t, c] = _FORMS[t][c]

    g_of_slot = np.empty((NCORES, GPC), dtype=np.int64)
    idx_maps = []
    hb = NBLK // 2
    ncopy = NBLK - naox
    for k in range(NCORES):
        segs, segcls = [], []
        for c in range(4):
            gk = np.asarray(assign[k][c], dtype=np.int64)
            if not len(gk):
                continue
            # sort by p-row for DRAM locality
            pr = _sec_rows(psec_tab[tt[gk], c], ch[gk, 0], ch[gk, 1])
            o = np.argsort(pr, kind="stable")
            segs.append(gk[o])
            segcls.append(np.full(len(gk), c))
        gk = np.concatenate(segs)
        cls = np.concatenate(segcls)
        assert gk.shape == (GPC,)
        g_of_slot[k] = gk
        p_rows = _sec_rows(psec_tab[tt[gk], cls], ch[gk, 0], ch[gk, 1])
        q_rows = _sec_rows(qsec_tab[tt[gk], cls], ch[gk, 0], ch[gk, 1])

        cols = []
        for h in (0, 1):
            flat = p_rows[h * hb * 128 : (h + 1) * hb * 128].astype(np.int16)
            cols.append(np.tile(flat.reshape(-1, 16).T, (8, 1)))
        for h in (0, 1):
            lo, hi = h * hb, min((h + 1) * hb, naox)
            if lo < hi:
                flat = q_rows[lo * 128 : hi * 128].astype(np.int16)
                cols.append(np.tile(flat.reshape(-1, 16).T, (8, 1)))
        idx_maps.append(np.ascontiguousarray(np.concatenate(cols, axis=1)))

    return (na, no, nx), g_of_slot, idx_maps


def _build_tab(x):
    x8 = np.asarray(x, dtype=np.uint8)
    xp = np.packbits(x8, axis=0)              # [PB, N]
    tab = np.empty((NTAB, PB), dtype=np.uint8)
    tab[:N] = xp.T
    tab[N : 2 * N] = 255 - tab[:N]
    tab[2 * N] = 0
    tab[2 * N + 1] = 255
    return tab.view(np.uint16)


# ---------------------------------------------------------------------------
# Entry point
# ---------------------------------------------------------------------------

_PLAN_CACHE = {}


def _get_plan(gates, choices):
    h = hash((gates.tobytes(), choices.tobytes()))
    if h not in _PLAN_CACHE:
        _PLAN_CACHE[h] = _plan(gates, choices)
    return _PLAN_CACHE[h]


def kernel(x, gates, choices):
    aox, g_of_slot, idx_maps = _get_plan(np.asarray(gates), np.asarray(choices))
    tab = _build_tab(x)
    nc = _get_nc(aox)
    in_maps = [{"tab": tab, "idxs": idx_maps[k]} for k in range(NCORES)]
    res = run_bass_kernel_spmd(nc, in_maps, list(range(NCORES)))

    packed = np.empty((G, PB), dtype=np.uint8)
    for k in range(NCORES):
        packed[g_of_slot[k]] = res.results[k]["out"].view(np.uint8)
    out = np.unpackbits(np.ascontiguousarray(packed.T), axis=0)
    return out.view(np.bool_)
